# revision 29
# baseline (speedup 1.0000x reference)
"""Trainium2 Bass kernel for a CNF (FFJORD-style) dynamics step.

Computes, for each sample z_i of a batch B=131072 (D=8, H=128):
    x  = concat([z_i, t])
    h1 = tanh(x @ W1 + b1)
    h2 = tanh(h1 @ W2 + b2)
    dz_dt   = h2 @ W3 + b3
    dlogp   = -trace(d dz_dt / d z_i)

The Jacobian trace has the closed form (u = 1-h1^2, v = 1-h2^2):
    trace = v . (u @ C)   with C[j,k] = W2[j,k] * (W3 @ W1[:D])[k,j]
so a single extra HxH matmul per sample replaces the full Jacobian.
On device both "1 -" terms are folded into constant weights:
    s'' = (-C)^T h1sq,   q = (s'' + c0) * h2sq   (fused DVE op)
    dlogp = crow . h1sq + sum_k q_k - S0
with c0 = C^T 1, crow = C 1, S0 = 1^T C 1 all precomputed on the host;
crow and the all-ones column are tiny extra stationary operands on the
PE, and -S0 rides the per-partition bias of the PSUM->SBUF output copy.

Sharding: pure data parallel over 8 NeuronCores (batch split).
Layout on device is feature-major ([feature, batch] in SBUF partitions);
the host transposes z per shard and transposes dz_dt back.

Tiles are processed in pairs: layer-1 matmuls of the two tiles run
concurrently in different 32-row groups of the PE array, activations and
elementwise ops run once per pair at free-dim 1024, and the dz (M=8) /
dlogp (M=1) matmuls of both tiles pack into the four 32-col groups of
one shared PSUM output bank.

The PE clock is activity-gated (1.2 GHz cold / 2.4 GHz warm); dep-free
filler matmuls into a scratch PSUM bank keep the array busy through
pipeline bubbles so it holds the warm clock.
"""

import numpy as np
import ml_dtypes

import concourse.bass as bass
import concourse.tile as tile
from concourse import bacc, mybir
from concourse.bass_utils import run_bass_kernel_spmd

BF16 = ml_dtypes.bfloat16

B = 131072
D = 8
H = 128
NCORES = 8
BC = B // NCORES          # samples per core
FD = 512                  # tile free-dim (samples per tile)
NTILES = BC // FD         # 32
GROUP = 2                 # tiles per pair-group
NG = NTILES // GROUP
ZBLK = 4                  # pair-groups per z-load DMA

# bf16 weights packed as one [128, WCOLS] image:
#   w2 | cneg | w3 | ones_pos | crow | w1r
WC_W2 = 0
WC_CN = H
WC_W3 = 2 * H
WC_ON = 2 * H + D
WC_CR = 2 * H + D + 1
WC_W1 = 2 * H + D + 2
WCOLS = WC_W1 + H

# test.py can read profiling info from here after calling kernel()
LAST_RESULTS = None


def _build_bass(with_b3):
    nc = bacc.Bacc("TRN2", target_bir_lowering=False, debug=False,
                   num_devices=NCORES)
    f32 = mybir.dt.float32
    bf16 = mybir.dt.bfloat16
    FD2 = FD * GROUP

    zta_d = nc.dram_tensor("zta", [D, BC // 2], bf16, kind="ExternalInput").ap()
    ztb_d = nc.dram_tensor("ztb", [D, BC // 2], bf16, kind="ExternalInput").ap()
    wb_d = nc.dram_tensor("wbig", [H, WCOLS], bf16, kind="ExternalInput").ap()
    bias_d = nc.dram_tensor("biases", [H, 4], f32, kind="ExternalInput").ap()

    dzt_d = nc.dram_tensor("dzt", [D, BC], f32, kind="ExternalOutput").ap()
    dlp_d = nc.dram_tensor("dlp", [NTILES, FD], f32, kind="ExternalOutput").ap()

    mult = mybir.AluOpType.mult
    add = mybir.AluOpType.add
    Tanh = mybir.ActivationFunctionType.Tanh
    Square = mybir.ActivationFunctionType.Square

    with tile.TileContext(nc) as tc:
        with (
            tc.tile_pool(name="wts", bufs=1) as wp,
            tc.tile_pool(name="io", bufs=8) as iop,
            tc.tile_pool(name="zt", bufs=2) as ztp_pool,
            tc.tile_pool(name="act", bufs=6) as ap_,
            tc.tile_pool(name="pa1", bufs=1, space="PSUM") as pa1,
            tc.tile_pool(name="pa2", bufs=1, space="PSUM") as pa2,
            tc.tile_pool(name="psm", bufs=1, space="PSUM") as psm,
            tc.tile_pool(name="pout", bufs=1, space="PSUM") as pout,
            tc.tile_pool(name="pfill", bufs=1, space="PSUM") as pfill,
        ):
            # scratch operands for PE-warming filler matmuls (content junk)
            junk = wp.tile([H, FD], bf16)
            nc.vector.memset(junk[:], 0.0)
            fill_bank = pfill.tile([H, FD], f32)

            def filler(n):
                for _ in range(n):
                    nc.tensor.matmul(fill_bank[0:24, :], junk[:, 0:24],
                                     junk[:], skip_group_check=True)

            wb = wp.tile([H, WCOLS], bf16)
            nc.sync.dma_start(wb[:], wb_d)
            biases = wp.tile([H, 4], f32)
            nc.sync.dma_start(biases[:], bias_d)
            w2 = wb[:, WC_W2:WC_W2 + H]
            cn = wb[:, WC_CN:WC_CN + H]
            w3 = wb[:, WC_W3:WC_W3 + D]
            ones = wb[:, WC_ON:WC_ON + 1]
            crow = wb[:, WC_CR:WC_CR + 1]
            w1r = wb[:, WC_W1:WC_W1 + H]   # rows 0-7 / 32-39 hold W1z
            b1p = biases[:, 0:1]
            b2 = biases[:, 1:2]
            b3p = biases[:, 2:3]
            c0 = biases[:, 3:4]

            # warm the PE while the weight/z DMAs are in flight
            filler(18)

            ztp = None
            for tg in range(NG):
                t0 = tg * GROUP
                colset = [bass.ts(t0 + j, FD) for j in range(GROUP)]

                if tg % ZBLK == 0:
                    # rows 0-7: even tiles, rows 32-39: odd tiles
                    ztp = ztp_pool.tile([32 + D, ZBLK * FD], bf16, tag="zt",
                                        name=f"ztp_{tg}")
                    zcols = bass.ts(tg // ZBLK, ZBLK * FD)
                    nc.sync.dma_start(ztp[0:D, :], zta_d[:, zcols])
                    nc.sync.dma_start(ztp[32:32 + D, :], ztb_d[:, zcols])
                zoff = (tg % ZBLK) * FD

                # layer-1 matmuls packed in row groups 0/1, one 2-bank out
                a1p = pa1.tile([H, FD2], f32, tag="a1")
                for j in range(GROUP):
                    nc.tensor.matmul(a1p[:, bass.ts(j, FD)],
                                     w1r[32 * j:32 * j + D, :],
                                     ztp[32 * j:32 * j + D,
                                         zoff:zoff + FD],
                                     tile_position=(32 * j, 0))
                filler(2)

                h1p = ap_.tile([H, FD2], bf16, tag="h1")
                nc.scalar.activation(h1p[:], a1p[:], Tanh, bias=b1p)
                h1sqp = ap_.tile([H, FD2], bf16, tag="h1sq")
                nc.vector.tensor_tensor(h1sqp[:], h1p[:], h1p[:], mult)

                a2p = pa2.tile([H, FD2], f32, tag="a2")
                sp = psm.tile([H, FD2], f32, tag="s")
                for j in range(GROUP):
                    nc.tensor.matmul(a2p[:, bass.ts(j, FD)], w2,
                                     h1p[:, bass.ts(j, FD)])
                    nc.tensor.matmul(sp[:, bass.ts(j, FD)], cn,
                                     h1sqp[:, bass.ts(j, FD)])
                filler(2)

                h2p = ap_.tile([H, FD2], bf16, tag="h2")
                nc.scalar.activation(h2p[:], a2p[:], Tanh, bias=b2)
                h2sqp = ap_.tile([H, FD2], bf16, tag="h2sq")
                if tg % 2 == 1:
                    nc.scalar.activation(h2sqp[:], h2p[:], Square)
                else:
                    nc.vector.tensor_tensor(h2sqp[:], h2p[:], h2p[:], mult)

                # q = (s'' + c0) * h2sq in one fused DVE op
                qp = ap_.tile([H, FD2], bf16, tag="q")
                nc.vector.scalar_tensor_tensor(qp[:], sp[:], c0, h2sqp[:],
                                               add, mult)

                out_bank = pout.tile([H, FD], f32, tag="ob")
                for j in range(GROUP):
                    nc.tensor.matmul(out_bank[32 * j:32 * j + D, :], w3,
                                     h2p[:, bass.ts(j, FD)],
                                     tile_position=(0, 32 * j))
                    nc.tensor.matmul(out_bank[64 + 32 * j:64 + 32 * j + 1, :],
                                     crow, h1sqp[:, bass.ts(j, FD)],
                                     start=True, stop=False,
                                     tile_position=(0, 64 + 32 * j),
                                     skip_group_check=True)
                for j in range(GROUP):
                    nc.tensor.matmul(out_bank[64 + 32 * j:64 + 32 * j + 1, :],
                                     ones, qp[:, bass.ts(j, FD)],
                                     start=False, stop=True,
                                     tile_position=(0, 64 + 32 * j),
                                     skip_group_check=True)
                filler(2)

                osb = iop.tile([H, FD], f32, tag="osb")
                nc.vector.tensor_scalar(osb[:], out_bank[:], b3p, None, add)
                for j in range(GROUP):
                    nc.sync.dma_start(dzt_d[:, colset[j]],
                                      osb[32 * j:32 * j + D, :])
                nc.scalar.dma_start(dlp_d[t0:t0 + GROUP, :],
                                    osb[64:64 + 32 * GROUP:32, :])
                if tg >= NG - 3:
                    filler(3)
    nc.compile()
    return nc


_NC_CACHE = {}


def kernel(z, logp_z, t, W1, b1, W2, b2, W3, b3):
    global LAST_RESULTS, _NC_CACHE

    z = np.ascontiguousarray(np.asarray(z, dtype=np.float32))
    t_s = float(np.asarray(t, dtype=np.float32).reshape(-1)[0])
    W1 = np.asarray(W1, dtype=np.float32)
    b1 = np.asarray(b1, dtype=np.float32)
    W2 = np.asarray(W2, dtype=np.float32)
    b2 = np.asarray(b2, dtype=np.float32)
    W3 = np.asarray(W3, dtype=np.float32)
    b3 = np.asarray(b3, dtype=np.float32)

    W1z = np.ascontiguousarray(W1[:D, :])              # [D, H]
    b1p = (b1 + t_s * W1[D, :]).astype(np.float32)
    M = W3.astype(np.float64) @ W1z.astype(np.float64)  # [H, H]
    C = W2.astype(np.float64) * M.T                     # [H(j), H(k)]
    c0 = C.sum(axis=0).astype(np.float32)               # C^T @ 1

    wbig = np.zeros((H, WCOLS), dtype=BF16)
    wbig[:, WC_W2:WC_W2 + H] = W2.astype(BF16)
    wbig[:, WC_CN:WC_CN + H] = (-C).astype(np.float32).astype(BF16)
    wbig[:, WC_W3:WC_W3 + D] = W3.astype(BF16)
    wbig[:, WC_ON] = 1.0
    crow = C.sum(axis=1).astype(np.float32)             # C @ 1
    wbig[:, WC_CR] = crow.astype(BF16)
    for j in range(GROUP):
        wbig[32 * j:32 * j + D, WC_W1:WC_W1 + H] = W1z.astype(BF16)

    S0 = float(C.sum())
    biases = np.zeros((H, 4), dtype=np.float32)
    biases[:, 0] = b1p
    biases[:, 1] = b2
    for jj in range(GROUP):
        biases[32 * jj:32 * jj + D, 2] = b3
        biases[64 + 32 * jj, 2] = -S0
    biases[:, 3] = c0
    with_b3 = bool(np.any(b3 != 0))

    if with_b3 not in _NC_CACHE:
        _NC_CACHE[with_b3] = _build_bass(with_b3)
    nc = _NC_CACHE[with_b3]

    in_maps = []
    for c in range(NCORES):
        zt = z[c * BC:(c + 1) * BC, :].T.astype(BF16)    # [D, BC]
        zt3 = zt.reshape(D, NTILES, FD)
        zta = np.ascontiguousarray(zt3[:, 0::2, :].reshape(D, BC // 2))
        ztb = np.ascontiguousarray(zt3[:, 1::2, :].reshape(D, BC // 2))
        in_maps.append({"zta": zta, "ztb": ztb, "wbig": wbig,
                        "biases": biases})

    res = run_bass_kernel_spmd(nc, in_maps, core_ids=list(range(NCORES)))
    LAST_RESULTS = res

    dz = np.empty((B, D), dtype=np.float32)
    dlogp = np.empty((B, 1), dtype=np.float32)
    for c in range(NCORES):
        dz[c * BC:(c + 1) * BC, :] = res.results[c]["dzt"].T
        dlogp[c * BC:(c + 1) * BC, 0] = res.results[c]["dlp"].reshape(-1)
    return (dz, dlogp)


# revision 30
# speedup vs baseline: 1.0122x; 1.0122x over previous
"""Trainium2 Bass kernel for a CNF (FFJORD-style) dynamics step.

Computes, for each sample z_i of a batch B=131072 (D=8, H=128):
    x  = concat([z_i, t])
    h1 = tanh(x @ W1 + b1)
    h2 = tanh(h1 @ W2 + b2)
    dz_dt   = h2 @ W3 + b3
    dlogp   = -trace(d dz_dt / d z_i)

The Jacobian trace has the closed form (u = 1-h1^2, v = 1-h2^2):
    trace = v . (u @ C)   with C[j,k] = W2[j,k] * (W3 @ W1[:D])[k,j]
so a single extra HxH matmul per sample replaces the full Jacobian.
On device both "1 -" terms are folded into constant weights:
    s'' = (-C)^T h1sq,   q = (s'' + c0) * h2sq   (fused DVE op)
    dlogp = crow . h1sq + sum_k q_k - S0
with c0 = C^T 1, crow = C 1, S0 = 1^T C 1 all precomputed on the host;
crow and the all-ones column are tiny extra stationary operands on the
PE, and -S0 rides the per-partition bias of the PSUM->SBUF output copy.

Sharding: pure data parallel over 8 NeuronCores (batch split).
Layout on device is feature-major ([feature, batch] in SBUF partitions);
the host transposes z per shard and transposes dz_dt back.

Tiles are processed in pairs: layer-1 matmuls of the two tiles run
concurrently in different 32-row groups of the PE array, activations and
elementwise ops run once per pair at free-dim 1024, and the dz (M=8) /
dlogp (M=1) matmuls of both tiles pack into the four 32-col groups of
one shared PSUM output bank.

The PE clock is activity-gated (1.2 GHz cold / 2.4 GHz warm); dep-free
filler matmuls into a scratch PSUM bank keep the array busy through
pipeline bubbles so it holds the warm clock.
"""

import numpy as np
import ml_dtypes

import concourse.bass as bass
import concourse.tile as tile
from concourse import bacc, mybir
from concourse.bass_utils import run_bass_kernel_spmd

BF16 = ml_dtypes.bfloat16

B = 131072
D = 8
H = 128
NCORES = 8
BC = B // NCORES          # samples per core
FD = 512                  # tile free-dim (samples per tile)
NTILES = BC // FD         # 32
GROUP = 2                 # tiles per pair-group
NG = NTILES // GROUP
ZBLK = 4                  # pair-groups per z-load DMA

# bf16 weights packed as one [128, WCOLS] image:
#   w2 | cneg | w3 | ones_pos | crow | w1r
WC_W2 = 0
WC_CN = H
WC_W3 = 2 * H
WC_ON = 2 * H + D
WC_CR = 2 * H + D + 1
WC_W1 = 2 * H + D + 2
WCOLS = WC_W1 + H

# test.py can read profiling info from here after calling kernel()
LAST_RESULTS = None


def _build_bass(with_b3):
    nc = bacc.Bacc("TRN2", target_bir_lowering=False, debug=False,
                   num_devices=NCORES)
    f32 = mybir.dt.float32
    bf16 = mybir.dt.bfloat16
    FD2 = FD * GROUP

    zta_d = nc.dram_tensor("zta", [D, BC // 2], bf16, kind="ExternalInput").ap()
    ztb_d = nc.dram_tensor("ztb", [D, BC // 2], bf16, kind="ExternalInput").ap()
    wb_d = nc.dram_tensor("wbig", [H, WCOLS], bf16, kind="ExternalInput").ap()
    bias_d = nc.dram_tensor("biases", [H, 4], f32, kind="ExternalInput").ap()

    dzt_d = nc.dram_tensor("dzt", [D, BC], f32, kind="ExternalOutput").ap()
    dlp_d = nc.dram_tensor("dlp", [NTILES, FD], f32, kind="ExternalOutput").ap()

    mult = mybir.AluOpType.mult
    add = mybir.AluOpType.add
    Tanh = mybir.ActivationFunctionType.Tanh
    Square = mybir.ActivationFunctionType.Square

    with tile.TileContext(nc) as tc:
        with (
            tc.tile_pool(name="wts", bufs=1) as wp,
            tc.tile_pool(name="io", bufs=8) as iop,
            tc.tile_pool(name="zt", bufs=2) as ztp_pool,
            tc.tile_pool(name="act", bufs=6) as ap_,
            tc.tile_pool(name="pa1", bufs=1, space="PSUM") as pa1,
            tc.tile_pool(name="pa2", bufs=1, space="PSUM") as pa2,
            tc.tile_pool(name="psm", bufs=1, space="PSUM") as psm,
            tc.tile_pool(name="pout", bufs=1, space="PSUM") as pout,
            tc.tile_pool(name="pfill", bufs=1, space="PSUM") as pfill,
        ):
            # scratch operands for PE-warming filler matmuls (content junk)
            junk = wp.tile([H, FD], bf16)
            nc.vector.memset(junk[:], 0.0)
            fill_bank = pfill.tile([H, FD], f32)

            def filler(n):
                for _ in range(n):
                    nc.tensor.matmul(fill_bank[0:24, :], junk[:, 0:24],
                                     junk[:], skip_group_check=True)

            wb = wp.tile([H, WCOLS], bf16)
            nc.sync.dma_start(wb[:], wb_d)
            biases = wp.tile([H, 4], f32)
            nc.sync.dma_start(biases[:], bias_d)
            w2 = wb[:, WC_W2:WC_W2 + H]
            cn = wb[:, WC_CN:WC_CN + H]
            w3 = wb[:, WC_W3:WC_W3 + D]
            ones = wb[:, WC_ON:WC_ON + 1]
            crow = wb[:, WC_CR:WC_CR + 1]
            w1r = wb[:, WC_W1:WC_W1 + H]   # rows 0-7 / 32-39 hold W1z
            b1p = biases[:, 0:1]
            b2 = biases[:, 1:2]
            b3p = biases[:, 2:3]
            c0 = biases[:, 3:4]

            # warm the PE while the weight/z DMAs are in flight
            filler(18)

            def load_zblk(blk):
                zt = ztp_pool.tile([32 + D, ZBLK * FD], bf16, tag="zt",
                                   name=f"ztp_{blk}")
                zcols = bass.ts(blk, ZBLK * FD)
                nc.sync.dma_start(zt[0:D, :], zta_d[:, zcols])
                nc.sync.dma_start(zt[32:32 + D, :], ztb_d[:, zcols])
                return zt

            ztp = load_zblk(0)
            ztp_next = None
            for tg in range(NG):
                t0 = tg * GROUP
                colset = [bass.ts(t0 + j, FD) for j in range(GROUP)]

                if tg % ZBLK == 1 and tg // ZBLK + 1 < NG // ZBLK:
                    ztp_next = load_zblk(tg // ZBLK + 1)
                elif tg % ZBLK == 0 and tg > 0:
                    ztp = ztp_next
                zoff = (tg % ZBLK) * FD

                # layer-1 matmuls packed in row groups 0/1, one 2-bank out
                a1p = pa1.tile([H, FD2], f32, tag="a1")
                for j in range(GROUP):
                    nc.tensor.matmul(a1p[:, bass.ts(j, FD)],
                                     w1r[32 * j:32 * j + D, :],
                                     ztp[32 * j:32 * j + D,
                                         zoff:zoff + FD],
                                     tile_position=(32 * j, 0))
                filler(2)

                h1p = ap_.tile([H, FD2], bf16, tag="h1")
                nc.scalar.activation(h1p[:], a1p[:], Tanh, bias=b1p)
                h1sqp = ap_.tile([H, FD2], bf16, tag="h1sq")
                nc.vector.tensor_tensor(h1sqp[:], h1p[:], h1p[:], mult)

                a2p = pa2.tile([H, FD2], f32, tag="a2")
                sp = psm.tile([H, FD2], f32, tag="s")
                for j in range(GROUP):
                    nc.tensor.matmul(a2p[:, bass.ts(j, FD)], w2,
                                     h1p[:, bass.ts(j, FD)])
                    nc.tensor.matmul(sp[:, bass.ts(j, FD)], cn,
                                     h1sqp[:, bass.ts(j, FD)])
                filler(2)

                h2p = ap_.tile([H, FD2], bf16, tag="h2")
                nc.scalar.activation(h2p[:], a2p[:], Tanh, bias=b2)
                h2sqp = ap_.tile([H, FD2], bf16, tag="h2sq")
                if tg % 2 == 0:
                    nc.scalar.activation(h2sqp[:], h2p[:], Square)
                else:
                    nc.vector.tensor_tensor(h2sqp[:], h2p[:], h2p[:], mult)

                # q = (s'' + c0) * h2sq in one fused DVE op
                qp = ap_.tile([H, FD2], bf16, tag="q")
                nc.vector.scalar_tensor_tensor(qp[:], sp[:], c0, h2sqp[:],
                                               add, mult)

                out_bank = pout.tile([H, FD], f32, tag="ob")
                for j in range(GROUP):
                    nc.tensor.matmul(out_bank[32 * j:32 * j + D, :], w3,
                                     h2p[:, bass.ts(j, FD)],
                                     tile_position=(0, 32 * j))
                    nc.tensor.matmul(out_bank[64 + 32 * j:64 + 32 * j + 1, :],
                                     crow, h1sqp[:, bass.ts(j, FD)],
                                     start=True, stop=False,
                                     tile_position=(0, 64 + 32 * j),
                                     skip_group_check=True)
                for j in range(GROUP):
                    nc.tensor.matmul(out_bank[64 + 32 * j:64 + 32 * j + 1, :],
                                     ones, qp[:, bass.ts(j, FD)],
                                     start=False, stop=True,
                                     tile_position=(0, 64 + 32 * j),
                                     skip_group_check=True)
                filler(2)

                osb = iop.tile([H, FD], f32, tag="osb")
                nc.vector.tensor_scalar(osb[:], out_bank[:], b3p, None, add)
                for j in range(GROUP):
                    nc.sync.dma_start(dzt_d[:, colset[j]],
                                      osb[32 * j:32 * j + D, :])
                nc.scalar.dma_start(dlp_d[t0:t0 + GROUP, :],
                                    osb[64:64 + 32 * GROUP:32, :])
                if tg >= NG - 3:
                    filler(3)
    nc.compile()
    return nc


_NC_CACHE = {}


def kernel(z, logp_z, t, W1, b1, W2, b2, W3, b3):
    global LAST_RESULTS, _NC_CACHE

    z = np.ascontiguousarray(np.asarray(z, dtype=np.float32))
    t_s = float(np.asarray(t, dtype=np.float32).reshape(-1)[0])
    W1 = np.asarray(W1, dtype=np.float32)
    b1 = np.asarray(b1, dtype=np.float32)
    W2 = np.asarray(W2, dtype=np.float32)
    b2 = np.asarray(b2, dtype=np.float32)
    W3 = np.asarray(W3, dtype=np.float32)
    b3 = np.asarray(b3, dtype=np.float32)

    W1z = np.ascontiguousarray(W1[:D, :])              # [D, H]
    b1p = (b1 + t_s * W1[D, :]).astype(np.float32)
    M = W3.astype(np.float64) @ W1z.astype(np.float64)  # [H, H]
    C = W2.astype(np.float64) * M.T                     # [H(j), H(k)]
    c0 = C.sum(axis=0).astype(np.float32)               # C^T @ 1

    wbig = np.zeros((H, WCOLS), dtype=BF16)
    wbig[:, WC_W2:WC_W2 + H] = W2.astype(BF16)
    wbig[:, WC_CN:WC_CN + H] = (-C).astype(np.float32).astype(BF16)
    wbig[:, WC_W3:WC_W3 + D] = W3.astype(BF16)
    wbig[:, WC_ON] = 1.0
    crow = C.sum(axis=1).astype(np.float32)             # C @ 1
    wbig[:, WC_CR] = crow.astype(BF16)
    for j in range(GROUP):
        wbig[32 * j:32 * j + D, WC_W1:WC_W1 + H] = W1z.astype(BF16)

    S0 = float(C.sum())
    biases = np.zeros((H, 4), dtype=np.float32)
    biases[:, 0] = b1p
    biases[:, 1] = b2
    for jj in range(GROUP):
        biases[32 * jj:32 * jj + D, 2] = b3
        biases[64 + 32 * jj, 2] = -S0
    biases[:, 3] = c0
    with_b3 = bool(np.any(b3 != 0))

    if with_b3 not in _NC_CACHE:
        _NC_CACHE[with_b3] = _build_bass(with_b3)
    nc = _NC_CACHE[with_b3]

    in_maps = []
    for c in range(NCORES):
        zt = z[c * BC:(c + 1) * BC, :].T.astype(BF16)    # [D, BC]
        zt3 = zt.reshape(D, NTILES, FD)
        zta = np.ascontiguousarray(zt3[:, 0::2, :].reshape(D, BC // 2))
        ztb = np.ascontiguousarray(zt3[:, 1::2, :].reshape(D, BC // 2))
        in_maps.append({"zta": zta, "ztb": ztb, "wbig": wbig,
                        "biases": biases})

    res = run_bass_kernel_spmd(nc, in_maps, core_ids=list(range(NCORES)))
    LAST_RESULTS = res

    dz = np.empty((B, D), dtype=np.float32)
    dlogp = np.empty((B, 1), dtype=np.float32)
    for c in range(NCORES):
        dz[c * BC:(c + 1) * BC, :] = res.results[c]["dzt"].T
        dlogp[c * BC:(c + 1) * BC, 0] = res.results[c]["dlp"].reshape(-1)
    return (dz, dlogp)


# revision 31
# speedup vs baseline: 1.0460x; 1.0334x over previous
"""Trainium2 Bass kernel for a CNF (FFJORD-style) dynamics step.

Computes, for each sample z_i of a batch B=131072 (D=8, H=128):
    x  = concat([z_i, t])
    h1 = tanh(x @ W1 + b1)
    h2 = tanh(h1 @ W2 + b2)
    dz_dt   = h2 @ W3 + b3
    dlogp   = -trace(d dz_dt / d z_i)

The Jacobian trace has the closed form (u = 1-h1^2, v = 1-h2^2):
    trace = v . (u @ C)   with C[j,k] = W2[j,k] * (W3 @ W1[:D])[k,j]
so a single extra HxH matmul per sample replaces the full Jacobian.
On device both "1 -" terms are folded into constant weights:
    s'' = (-C)^T h1sq,   q = (s'' + c0) * h2sq   (fused DVE op)
    dlogp = crow . h1sq + sum_k q_k - S0
with c0 = C^T 1, crow = C 1, S0 = 1^T C 1 all precomputed on the host;
crow and the all-ones column are tiny extra stationary operands on the
PE, and -S0 rides the per-partition bias of the PSUM->SBUF output copy.

Sharding: pure data parallel over 8 NeuronCores (batch split).
Layout on device is feature-major ([feature, batch] in SBUF partitions);
the host transposes z per shard and transposes dz_dt back.

Tiles are processed in pairs: layer-1 matmuls of the two tiles run
concurrently in different 32-row groups of the PE array, activations and
elementwise ops run once per pair at free-dim 1024, and the dz (M=8) /
dlogp (M=1) matmuls of both tiles pack into the four 32-col groups of
one shared PSUM output bank.

The PE clock is activity-gated (1.2 GHz cold / 2.4 GHz warm); dep-free
filler matmuls into a scratch PSUM bank keep the array busy through
pipeline bubbles so it holds the warm clock.
"""

import numpy as np
import ml_dtypes

import concourse.bass as bass
import concourse.tile as tile
from concourse import bacc, mybir
from concourse.bass_utils import run_bass_kernel_spmd

BF16 = ml_dtypes.bfloat16

B = 131072
D = 8
H = 128
NCORES = 8
BC = B // NCORES          # samples per core
FD = 512                  # tile free-dim (samples per tile)
NTILES = BC // FD         # 32
GROUP = 2                 # tiles per pair-group
NG = NTILES // GROUP
ZBLK = 4                  # pair-groups per z-load DMA

# bf16 weights packed as one [128, WCOLS] image:
#   w2 | cneg | w3 | ones_pos | crow | w1r
WC_W2 = 0
WC_CN = H
WC_W3 = 2 * H
WC_ON = 2 * H + D
WC_CR = 2 * H + D + 1
WC_W1 = 2 * H + D + 2
WCOLS = WC_W1 + H

# test.py can read profiling info from here after calling kernel()
LAST_RESULTS = None


def _build_bass(with_b3):
    nc = bacc.Bacc("TRN2", target_bir_lowering=False, debug=False,
                   num_devices=NCORES)
    f32 = mybir.dt.float32
    bf16 = mybir.dt.bfloat16
    FD2 = FD * GROUP

    zta_d = nc.dram_tensor("zta", [D, BC // 2], bf16, kind="ExternalInput").ap()
    ztb_d = nc.dram_tensor("ztb", [D, BC // 2], bf16, kind="ExternalInput").ap()
    wb_d = nc.dram_tensor("wbig", [H, WCOLS], bf16, kind="ExternalInput").ap()
    bias_d = nc.dram_tensor("biases", [H, 4], f32, kind="ExternalInput").ap()

    dzt_d = nc.dram_tensor("dzt", [D, BC], f32, kind="ExternalOutput").ap()
    dlp_d = nc.dram_tensor("dlp", [NTILES, FD], f32, kind="ExternalOutput").ap()

    mult = mybir.AluOpType.mult
    add = mybir.AluOpType.add
    Tanh = mybir.ActivationFunctionType.Tanh
    Square = mybir.ActivationFunctionType.Square

    with tile.TileContext(nc) as tc:
        with (
            tc.tile_pool(name="wts", bufs=1) as wp,
            tc.tile_pool(name="io", bufs=8) as iop,
            tc.tile_pool(name="zt", bufs=2) as ztp_pool,
            tc.tile_pool(name="act", bufs=6) as ap_,
            tc.tile_pool(name="pa1", bufs=1, space="PSUM") as pa1,
            tc.tile_pool(name="pa2", bufs=1, space="PSUM") as pa2,
            tc.tile_pool(name="psm", bufs=1, space="PSUM") as psm,
            tc.tile_pool(name="pout", bufs=1, space="PSUM") as pout,
            tc.tile_pool(name="pfill", bufs=1, space="PSUM") as pfill,
        ):
            # scratch operands for PE-warming filler matmuls (content junk)
            junk = wp.tile([H, FD], bf16)
            nc.vector.memset(junk[:], 0.0)
            fill_bank = pfill.tile([H, FD], f32)

            def filler(n):
                for _ in range(n):
                    nc.tensor.matmul(fill_bank[0:24, :], junk[:, 0:24],
                                     junk[:], skip_group_check=True)

            wb = wp.tile([H, WCOLS], bf16)
            nc.sync.dma_start(wb[:], wb_d)
            biases = wp.tile([H, 4], f32)
            nc.sync.dma_start(biases[:], bias_d)
            w2 = wb[:, WC_W2:WC_W2 + H]
            cn = wb[:, WC_CN:WC_CN + H]
            w3 = wb[:, WC_W3:WC_W3 + D]
            ones = wb[:, WC_ON:WC_ON + 1]
            crow = wb[:, WC_CR:WC_CR + 1]
            w1r = wb[:, WC_W1:WC_W1 + H]   # rows 0-7 / 32-39 hold W1z
            b1p = biases[:, 0:1]
            b2 = biases[:, 1:2]
            b3p = biases[:, 2:3]
            c0 = biases[:, 3:4]

            # warm the PE while the weight/z DMAs are in flight
            filler(9)

            ztp = None
            for tg in range(NG):
                t0 = tg * GROUP
                colset = [bass.ts(t0 + j, FD) for j in range(GROUP)]

                if tg % ZBLK == 0:
                    # rows 0-7: even tiles, rows 32-39: odd tiles
                    ztp = ztp_pool.tile([32 + D, ZBLK * FD], bf16, tag="zt",
                                        name=f"ztp_{tg}")
                    zcols = bass.ts(tg // ZBLK, ZBLK * FD)
                    nc.sync.dma_start(ztp[0:D, :], zta_d[:, zcols])
                    nc.sync.dma_start(ztp[32:32 + D, :], ztb_d[:, zcols])
                zoff = (tg % ZBLK) * FD

                # layer-1 matmuls packed in row groups 0/1, one 2-bank out
                a1p = pa1.tile([H, FD2], f32, tag="a1")
                for j in range(GROUP):
                    nc.tensor.matmul(a1p[:, bass.ts(j, FD)],
                                     w1r[32 * j:32 * j + D, :],
                                     ztp[32 * j:32 * j + D,
                                         zoff:zoff + FD],
                                     tile_position=(32 * j, 0))
                filler(2)

                h1p = ap_.tile([H, FD2], bf16, tag="h1")
                nc.scalar.activation(h1p[:], a1p[:], Tanh, bias=b1p)
                h1sqp = ap_.tile([H, FD2], bf16, tag="h1sq")
                nc.vector.tensor_tensor(h1sqp[:], h1p[:], h1p[:], mult)

                a2p = pa2.tile([H, FD2], f32, tag="a2")
                sp = psm.tile([H, FD2], f32, tag="s")
                for j in range(GROUP):
                    nc.tensor.matmul(a2p[:, bass.ts(j, FD)], w2,
                                     h1p[:, bass.ts(j, FD)])
                    nc.tensor.matmul(sp[:, bass.ts(j, FD)], cn,
                                     h1sqp[:, bass.ts(j, FD)])
                filler(2)

                h2p = ap_.tile([H, FD2], bf16, tag="h2")
                nc.scalar.activation(h2p[:], a2p[:], Tanh, bias=b2)
                h2sqp = ap_.tile([H, FD2], bf16, tag="h2sq")
                if tg % 2 == 0:
                    nc.scalar.activation(h2sqp[:], h2p[:], Square)
                else:
                    nc.vector.tensor_tensor(h2sqp[:], h2p[:], h2p[:], mult)

                # q = (s'' + c0) * h2sq in one fused DVE op
                qp = ap_.tile([H, FD2], bf16, tag="q")
                nc.vector.scalar_tensor_tensor(qp[:], sp[:], c0, h2sqp[:],
                                               add, mult)

                out_bank = pout.tile([H, FD], f32, tag="ob")
                for j in range(GROUP):
                    nc.tensor.matmul(out_bank[32 * j:32 * j + D, :], w3,
                                     h2p[:, bass.ts(j, FD)],
                                     tile_position=(0, 32 * j))
                    nc.tensor.matmul(out_bank[64 + 32 * j:64 + 32 * j + 1, :],
                                     crow, h1sqp[:, bass.ts(j, FD)],
                                     start=True, stop=False,
                                     tile_position=(0, 64 + 32 * j),
                                     skip_group_check=True)
                for j in range(GROUP):
                    nc.tensor.matmul(out_bank[64 + 32 * j:64 + 32 * j + 1, :],
                                     ones, qp[:, bass.ts(j, FD)],
                                     start=False, stop=True,
                                     tile_position=(0, 64 + 32 * j),
                                     skip_group_check=True)
                filler(2)

                osb = iop.tile([H, FD], f32, tag="osb")
                nc.vector.tensor_scalar(osb[:], out_bank[:], b3p, None, add)
                for j in range(GROUP):
                    nc.sync.dma_start(dzt_d[:, colset[j]],
                                      osb[32 * j:32 * j + D, :])
                nc.scalar.dma_start(dlp_d[t0:t0 + GROUP, :],
                                    osb[64:64 + 32 * GROUP:32, :])
                if tg >= NG - 3:
                    filler(3)
    nc.compile()
    return nc


_NC_CACHE = {}


def kernel(z, logp_z, t, W1, b1, W2, b2, W3, b3):
    global LAST_RESULTS, _NC_CACHE

    z = np.ascontiguousarray(np.asarray(z, dtype=np.float32))
    t_s = float(np.asarray(t, dtype=np.float32).reshape(-1)[0])
    W1 = np.asarray(W1, dtype=np.float32)
    b1 = np.asarray(b1, dtype=np.float32)
    W2 = np.asarray(W2, dtype=np.float32)
    b2 = np.asarray(b2, dtype=np.float32)
    W3 = np.asarray(W3, dtype=np.float32)
    b3 = np.asarray(b3, dtype=np.float32)

    W1z = np.ascontiguousarray(W1[:D, :])              # [D, H]
    b1p = (b1 + t_s * W1[D, :]).astype(np.float32)
    M = W3.astype(np.float64) @ W1z.astype(np.float64)  # [H, H]
    C = W2.astype(np.float64) * M.T                     # [H(j), H(k)]
    c0 = C.sum(axis=0).astype(np.float32)               # C^T @ 1

    wbig = np.zeros((H, WCOLS), dtype=BF16)
    wbig[:, WC_W2:WC_W2 + H] = W2.astype(BF16)
    wbig[:, WC_CN:WC_CN + H] = (-C).astype(np.float32).astype(BF16)
    wbig[:, WC_W3:WC_W3 + D] = W3.astype(BF16)
    wbig[:, WC_ON] = 1.0
    crow = C.sum(axis=1).astype(np.float32)             # C @ 1
    wbig[:, WC_CR] = crow.astype(BF16)
    for j in range(GROUP):
        wbig[32 * j:32 * j + D, WC_W1:WC_W1 + H] = W1z.astype(BF16)

    S0 = float(C.sum())
    biases = np.zeros((H, 4), dtype=np.float32)
    biases[:, 0] = b1p
    biases[:, 1] = b2
    for jj in range(GROUP):
        biases[32 * jj:32 * jj + D, 2] = b3
        biases[64 + 32 * jj, 2] = -S0
    biases[:, 3] = c0
    with_b3 = bool(np.any(b3 != 0))

    if with_b3 not in _NC_CACHE:
        _NC_CACHE[with_b3] = _build_bass(with_b3)
    nc = _NC_CACHE[with_b3]

    in_maps = []
    for c in range(NCORES):
        zt = z[c * BC:(c + 1) * BC, :].T.astype(BF16)    # [D, BC]
        zt3 = zt.reshape(D, NTILES, FD)
        zta = np.ascontiguousarray(zt3[:, 0::2, :].reshape(D, BC // 2))
        ztb = np.ascontiguousarray(zt3[:, 1::2, :].reshape(D, BC // 2))
        in_maps.append({"zta": zta, "ztb": ztb, "wbig": wbig,
                        "biases": biases})

    res = run_bass_kernel_spmd(nc, in_maps, core_ids=list(range(NCORES)))
    LAST_RESULTS = res

    dz = np.empty((B, D), dtype=np.float32)
    dlogp = np.empty((B, 1), dtype=np.float32)
    for c in range(NCORES):
        dz[c * BC:(c + 1) * BC, :] = res.results[c]["dzt"].T
        dlogp[c * BC:(c + 1) * BC, 0] = res.results[c]["dlp"].reshape(-1)
    return (dz, dlogp)
